# revision 27
# baseline (speedup 1.0000x reference)
"""Bass/Trainium2 kernel for nn_BipartiteSoftMatching (8 cores, batch-parallel).

Since r = t//2 the argsort in the reference is irrelevant: src_idx is a full
permutation and unm_idx is empty.  Per batch element the computation reduces to
  m = metric / ||metric||;  scores = m_even @ m_odd^T
  node_idx[i] = argmax_j scores[i, j]
  dst_out[j]  = (x_odd[j] + sum_{i: node_idx[i]=j} x_even[i]) / (1 + count[j])
  out[2j+1]   = dst_out[j];   out[2i] = dst_out[node_idx[i]]

Hardware mapping:
  - scores: fp32 matmuls, K=64 pairs row-packed via tile_position (exact
    enough; f32r would round operands to ~12 bits and flip argmaxes --
    measured min top-2 gap on this data is 2.7e-6).
  - argmax: DVE max (top-8) + max_index.
  - scatter-add, bucketed: tokens are permuted on-device into 16 buckets by
    destination j-tile (256 slots each, OOB-padded).  The permutation
    (slot = 256*bucket + cross-tile-count + within-tile-rank) is built with a
    transpose + lower-triangular-mask matmul trick, scattered to a DRAM
    staging table, and x_even rows are gathered back bucket-contiguous.
    Each j-tile then needs only 2 one-hot f32r matmuls per chunk/pass
    (vs 16) against x split into hi+lo f32r parts (2 passes ~ 2^-24 exact).
    A ones-column appended to x_hi yields the counts for free.
  - out even rows: per-tile indirect gathers of out's odd rows by
    2*node_idx+1, then one merged store.
"""

import numpy as np

import concourse.bacc as bacc
import concourse.bass as bass
import concourse.mybir as mybir
import concourse.tile as tile
from concourse.bass import IndirectOffsetOnAxis
from concourse.bass_utils import run_bass_kernel_spmd
from concourse.masks import make_identity

F32 = mybir.dt.float32
F32R = mybir.dt.float32r
U32 = mybir.dt.uint32
I32 = mybir.dt.int32
OP = mybir.AluOpType
AF = mybir.ActivationFunctionType

N, T, CM, CX = 8, 4096, 64, 768
P = 128
T1 = T // 2          # 2048 tokens per side
TI = T1 // P         # 16 i-tiles (even side)
TJ = T1 // P         # 16 j-tiles (odd side)
CXP = CX + 4         # x width padded: 768 data + 1 count col + 3 pad
NSLOT = 2 * T1       # 16 buckets x 256 slots
BIG = 1 << 20        # pad marker in the permutation table

_CACHE = {}


def _build(debug=False):
    nc = bacc.Bacc("TRN2", target_bir_lowering=False, num_devices=N)
    metric_in = nc.declare_dram_parameter("metric", [T, CM], F32, isOutput=False)
    x_in = nc.declare_dram_parameter("x", [T, CX], F32, isOutput=False)
    out = nc.declare_dram_parameter("out", [T, CX], F32, isOutput=True)
    if debug:
        perm8 = nc.declare_dram_parameter("perm8", [NSLOT, 2], I32, isOutput=True)
    else:
        perm8 = nc.dram_tensor("perm8", [NSLOT, 2], I32)

    # token = (t*128 + p)*2 + e
    m_pv = metric_in[:].rearrange("(t p e) c -> e p t c", p=P, e=2)
    x_r = x_in[:].rearrange("(t p e) c -> e t p c", p=P, e=2)
    out_r = out[:].rearrange("(t p e) c -> e t p c", p=P, e=2)
    out_pv = out[:].rearrange("(t p e) c -> e p t c", p=P, e=2)
    perm_pv = perm8[:].rearrange("(u p) w -> p u w", p=P)

    with tile.TileContext(nc, num_cores=N) as tc:
        with tc.tile_pool(name="const", bufs=1) as cp:
            ident = cp.tile([P, P], F32)
            make_identity(nc, ident[:])
            iota_row = cp.tile([P, T1], F32)
            nc.gpsimd.iota(iota_row[:], pattern=[[1, T1]], base=0,
                           channel_multiplier=0,
                           allow_small_or_imprecise_dtypes=True)
            iota16 = cp.tile([P, 16], F32)
            nc.gpsimd.iota(iota16[:], pattern=[[1, 16]], base=0,
                           channel_multiplier=0,
                           allow_small_or_imprecise_dtypes=True)
            ones128 = cp.tile([P, P], F32)
            nc.vector.memset(ones128[:], 1.0)
            # LT[p, f] = 1.0 if f < p else 0.0 (strict lower triangle)
            lt_i = cp.tile([P, P], I32)
            nc.gpsimd.iota(lt_i[:], pattern=[[1, P]], base=0,
                           channel_multiplier=-1)
            ltm = cp.tile([P, P], F32)
            nc.vector.tensor_scalar(ltm[:], lt_i[:], 0, None, op0=OP.is_lt)
            # xrow[p, t] = 2p + 256t = DRAM row of even token (t*128+p)
            xrow_i32 = cp.tile([P, TI], I32)
            nc.gpsimd.iota(xrow_i32[:], pattern=[[256, TI]], base=0,
                           channel_multiplier=2)
            cnt_pat = cp.tile([P, 4], F32)
            nc.vector.memset(cnt_pat[:], 0.0)
            nc.vector.memset(cnt_pat[:, 0:1], 1.0)
            bigpat = cp.tile([P, 2 * NSLOT // P], I32)
            nc.vector.memset(bigpat[:], BIG)

            idxf = cp.tile([P, TI], F32)
            offs = cp.tile([P, TI], I32)
            mi_all = cp.tile([P, TI * 8], U32)
            slot_f = cp.tile([P, TI], F32)
            slot_i32 = cp.tile([P, TI], I32)
            pr_all = cp.tile([P, TI * 2], I32)
            crun = cp.tile([1, 16], F32)       # running bucket counts
            nc.vector.memset(crun[:], 0.0)

            # pre-fill the permutation table with the OOB marker
            nc.sync.dma_start(out=perm8[:].rearrange("(p u) w -> p (u w)", p=P),
                              in_=bigpat[:])

            with tc.tile_pool(name="work", bufs=1) as wp:
                aTpk = wp.tile([P, T1 // 2], F32)
                bTpk = wp.tile([P, T1], F32)
                me = wp.tile([P, TI * CM], F32)
                mo = wp.tile([P, TI * CM], F32)

                nc.sync.dma_start(out=me[:].rearrange("p (t c) -> p t c", c=CM),
                                  in_=m_pv[0])
                nc.sync.dma_start(out=mo[:].rearrange("p (t c) -> p t c", c=CM),
                                  in_=m_pv[1])

                with tc.tile_pool(name="pA", bufs=3) as pa, \
                     tc.tile_pool(name="pB", bufs=2) as pb, \
                     tc.tile_pool(name="psA", bufs=2, space="PSUM") as psa, \
                     tc.tile_pool(name="psB", bufs=2, space="PSUM") as psb, \
                     tc.tile_pool(name="psR", bufs=1, space="PSUM") as psr:

                    # ---- Phase A: normalize metric, transpose directly into
                    # the packed operands (col-offset tile_position writes the
                    # upper partition half) ----
                    def normalize(src, t):
                        mt = src[:, t * CM:(t + 1) * CM]
                        sq = pa.tile([P, CM], F32, tag="sq")
                        ssum = pa.tile([P, 1], F32, tag="ss")
                        nc.scalar.activation(sq[:], mt, AF.Square,
                                             accum_out=ssum[:])
                        nrm = pa.tile([P, 1], F32, tag="nr")
                        nc.scalar.sqrt(nrm[:], ssum[:])
                        rnm = pa.tile([P, 1], F32, tag="rn")
                        nc.vector.reciprocal(rnm[:], nrm[:])
                        nm = pa.tile([P, CM], F32, tag="nm")
                        nc.vector.tensor_scalar_mul(nm[:], mt, rnm[:, 0:1])
                        return nm

                    for t in range(TI):          # odd side -> both bTpk halves
                        nm = normalize(mo, t)
                        pst = psa.tile([CM, P], F32, tag="tp", space="PSUM")
                        nc.tensor.transpose(pst[:], nm[:], ident[:])
                        blk = bTpk[:, t * P:(t + 1) * P]
                        nc.scalar.copy(blk[0:CM, :], pst[:])
                        nc.sync.dma_start(out=blk[CM:P, :], in_=blk[0:CM, :])
                    for t in range(TI):          # even side -> aTpk half by parity
                        nm = normalize(me, t)
                        pst = psa.tile([CM, P], F32, tag="tp", space="PSUM")
                        nc.tensor.transpose(pst[:], nm[:], ident[:])
                        blk = aTpk[:, (t // 2) * P:(t // 2 + 1) * P]
                        if t % 2 == 0:
                            nc.scalar.copy(blk[0:CM, :], pst[:])
                        else:
                            stg = pa.tile([CM, P], F32, tag="stg")
                            nc.scalar.copy(stg[:], pst[:])
                            nc.sync.dma_start(out=blk[CM:P, :], in_=stg[:])

                    # ---- Phase B: scores + argmax + slot computation ----
                    def rank_and_scatter(i):
                        """Per-tile permutation work: bucket, within-tile rank,
                        cross-tile count, slot, scatter (row, idx) to perm8."""
                        mi8 = mi_all[:, 8 * i:8 * i + 8]
                        nc.vector.tensor_copy(idxf[:, i:i + 1], mi8[:, 0:1])
                        nc.vector.tensor_copy(pr_all[:, 2 * i + 1:2 * i + 2],
                                              mi8[:, 0:1])
                        bu = pb.tile([P, 1], U32, tag="bu")
                        nc.vector.tensor_scalar(bu[:], mi8[:, 0:1], 7, None,
                                                op0=OP.logical_shift_right)
                        bf = pb.tile([P, 1], F32, tag="bf")
                        nc.vector.tensor_copy(bf[:], bu[:])
                        # one-hot over 16 buckets
                        oh = pb.tile([P, 16], F32, tag="oh")
                        nc.vector.tensor_scalar(oh[:], iota16[:], bf[:, 0:1],
                                                None, op0=OP.is_equal)
                        # within-tile rank: W[p] = #{p' < p: b[p'] == b[p]}
                        bt_ps = psr.tile([P, P], F32, tag="bt", space="PSUM")
                        nc.tensor.transpose(bt_ps[:], bf[:].to_broadcast([P, P]),
                                            ident[:])
                        bt_sb = pb.tile([P, P], F32, tag="bts")
                        nc.scalar.copy(bt_sb[:], bt_ps[:])
                        scr = pb.tile([P, P], F32, tag="scr")
                        wcol = pb.tile([P, 1], F32, tag="wc")
                        nc.vector.scalar_tensor_tensor(
                            out=scr[:], in0=bt_sb[:], scalar=bf[:, 0:1],
                            in1=ltm[:], op0=OP.is_equal, op1=OP.mult,
                            accum_out=wcol[:])
                        # cross-tile count of this bucket so far
                        cb = pb.tile([P, 16], F32, tag="cb")
                        nc.gpsimd.partition_broadcast(cb[:], crun[:])
                        ctv = pb.tile([P, 1], F32, tag="ctv")
                        s2 = pb.tile([P, 16], F32, tag="s2")
                        nc.vector.scalar_tensor_tensor(
                            out=s2[:], in0=oh[:], scalar=1.0, in1=cb[:],
                            op0=OP.mult, op1=OP.mult, accum_out=ctv[:])
                        # update running counts: crun += hist(b) (via matmul)
                        hp = psr.tile([1, 16], F32, tag="hp", space="PSUM")
                        nc.tensor.matmul(hp[:], ones128[:, 0:1], oh[:],
                                         start=True, stop=True)
                        nc.vector.tensor_add(crun[:], crun[:], hp[:])
                        # slot = 256*b + ctv + W
                        sf = slot_f[:, i:i + 1]
                        nc.vector.scalar_tensor_tensor(
                            out=sf, in0=bf[:], scalar=256.0, in1=ctv[:],
                            op0=OP.mult, op1=OP.add)
                        nc.vector.tensor_tensor(out=sf, in0=sf, in1=wcol[:],
                                                op=OP.add)
                        nc.vector.tensor_copy(slot_i32[:, i:i + 1], sf)
                        nc.vector.tensor_copy(pr_all[:, 2 * i:2 * i + 1],
                                              xrow_i32[:, i:i + 1])
                        nc.gpsimd.indirect_dma_start(
                            out=perm8[:], in_=pr_all[:, 2 * i:2 * i + 2],
                            in_offset=None,
                            out_offset=IndirectOffsetOnAxis(
                                ap=slot_i32[:, i:i + 1], axis=0))

                    for ii in range(TI // 2):
                        i0, i1 = 2 * ii, 2 * ii + 1
                        ssb0 = pb.tile([P, T1], F32, tag="scores0")
                        ssb1 = pb.tile([P, T1], F32, tag="scores1")
                        for c in range(4):
                            nj = c * 512
                            ps0 = psb.tile([P, 512], F32, tag="ps0", space="PSUM")
                            ps1 = psb.tile([P, 512], F32, tag="ps1", space="PSUM")
                            nc.tensor.matmul(ps0[:], aTpk[0:CM, ii * P:(ii + 1) * P],
                                             bTpk[0:CM, nj:nj + 512],
                                             start=True, stop=True,
                                             tile_position=(0, 0))
                            nc.tensor.matmul(ps1[:], aTpk[CM:P, ii * P:(ii + 1) * P],
                                             bTpk[CM:P, nj:nj + 512],
                                             start=True, stop=True,
                                             tile_position=(64, 0))
                            nc.scalar.copy(ssb0[:, nj:nj + 512], ps0[:])
                            nc.scalar.copy(ssb1[:, nj:nj + 512], ps1[:])
                        for i, ssb in ((i0, ssb0), (i1, ssb1)):
                            mx8 = pb.tile([P, 8], F32, tag="mx")
                            nc.vector.max(out=mx8[:], in_=ssb[:])
                            nc.vector.max_index(out=mi_all[:, 8 * i:8 * i + 8],
                                                in_max=mx8[:], in_values=ssb[:])
                            rank_and_scatter(i)

                    # gather offsets for phase D (odd row of out = 2*idx+1)
                    off_f = pb.tile([P, TI], F32)
                    nc.vector.tensor_scalar(off_f[:], idxf[:], 2.0, 1.0,
                                            op0=OP.mult, op1=OP.add)
                    nc.vector.tensor_copy(offs[:], off_f[:])

            # ---- Phase C: bucketed one-hot scatter matmul ----
            with tc.tile_pool(name="pq", bufs=1) as pqp, \
                 tc.tile_pool(name="pC", bufs=6) as pcs, \
                 tc.tile_pool(name="pD", bufs=2) as pd, \
                 tc.tile_pool(name="psC", bufs=2, space="PSUM") as psc:
                pq = pqp.tile([P, NSLOT // P * 2], I32)
                nc.sync.dma_start(
                    out=pq[:].rearrange("p (u w) -> p u w", w=2), in_=perm_pv)
                pq_v = pq[:].rearrange("p (u w) -> p u w", w=2)
                idxg_f = pqp.tile([P, NSLOT // P], F32)
                nc.vector.tensor_copy(idxg_f[:], pq_v[:, :, 1])
                # clamp pad-slot offsets (BIG) to a valid row; their one-hot
                # row is all-zero so the gathered data is multiplied away
                qoff = pqp.tile([P, NSLOT // P], I32)
                nc.vector.tensor_scalar(qoff[:], pq_v[:, :, 0], T - 2, None,
                                        op0=OP.min)

                ones_r = pqp.tile([P, 2], F32R)
                nc.vector.tensor_copy(ones_r[:], ones128[:, 0:2])
                for jt in range(TJ):
                    psj = psc.tile([P, CX], F32, tag="sp", space="PSUM")
                    psn = psc.tile([P, 2], F32, tag="sn", space="PSUM")
                    for k in range(2):
                        u = 2 * jt + k
                        xg = pcs.tile([P, CX], F32, tag="xg")
                        nc.gpsimd.indirect_dma_start(
                            out=xg[:], out_offset=None,
                            in_=x_in[:],
                            in_offset=IndirectOffsetOnAxis(
                                ap=qoff[:, u:u + 1], axis=0))
                        xh = pcs.tile([P, CX], F32R, tag="xh")
                        xl = pcs.tile([P, CX], F32R, tag="xl")
                        nc.vector.tensor_copy(xh[:], xg[:])
                        nc.gpsimd.tensor_tensor(out=xl[:], in0=xg[:],
                                                in1=xh[:].bitcast(F32),
                                                op=OP.subtract)
                        eqr = pcs.tile([P, P], F32R, tag="eq")
                        nc.vector.scalar_tensor_tensor(
                            out=eqr[:],
                            in0=iota_row[:, jt * P:(jt + 1) * P],
                            scalar=idxg_f[:, u:u + 1],
                            in1=ones128[:],
                            op0=OP.is_equal, op1=OP.mult)
                        first, last = (k == 0), (k == 1)
                        nc.tensor.matmul(psn[:], eqr[:], ones_r[:],
                                         start=first, stop=last)
                        for lo_, hi_ in ((0, 512), (512, CX)):
                            nc.tensor.matmul(psj[:, lo_:hi_], eqr[:],
                                             xh[:, lo_:hi_],
                                             start=first, stop=False)
                            nc.tensor.matmul(psj[:, lo_:hi_], eqr[:],
                                             xl[:, lo_:hi_],
                                             start=False, stop=last)
                    xo = pd.tile([P, CX], F32, tag="xo")
                    nc.sync.dma_start(out=xo[:], in_=x_r[1, jt])
                    cnt1 = pd.tile([P, 1], F32, tag="c1")
                    nc.vector.tensor_scalar_add(cnt1[:], psn[:, 0:1], 1.0)
                    inv = pd.tile([P, 1], F32, tag="iv")
                    nc.vector.reciprocal(inv[:], cnt1[:])
                    dst = pd.tile([P, CX], F32, tag="dst")
                    nc.vector.tensor_add(dst[:], xo[:], psj[:, 0:CX])
                    nc.scalar.mul(dst[:], dst[:], inv[:, 0:1])
                    nc.sync.dma_start(out=out_r[1, jt], in_=dst[:])

            # ---- Phase D: gather even rows, store in 4-tile chunks ----
            with tc.tile_pool(name="pG", bufs=1) as pg:
                gb = pg.tile([P, TI * CX], F32)
                gb_v = gb[:].rearrange("p (t c) -> p t c", c=CX)
                for i in range(TI):
                    nc.gpsimd.indirect_dma_start(
                        out=gb[:, i * CX:(i + 1) * CX], out_offset=None,
                        in_=out[:],
                        in_offset=IndirectOffsetOnAxis(ap=offs[:, i:i + 1], axis=0))
                    if i % 4 == 3:
                        nc.sync.dma_start(out=out_pv[0][:, i - 3:i + 1, :],
                                          in_=gb_v[:, i - 3:i + 1, :])

    nc.compile()
    return nc


def kernel(metric: np.ndarray, x: np.ndarray) -> np.ndarray:
    if "nc" not in _CACHE:
        _CACHE["nc"] = _build()
    nc = _CACHE["nc"]
    metric = np.ascontiguousarray(np.asarray(metric, dtype=np.float32))
    x = np.ascontiguousarray(np.asarray(x, dtype=np.float32))
    in_maps = [{"metric": metric[c], "x": x[c]} for c in range(N)]
    res = run_bass_kernel_spmd(nc, in_maps, list(range(N)))
    return np.stack([res.results[c]["out"] for c in range(N)], axis=0)


# revision 34
# speedup vs baseline: 1.0156x; 1.0156x over previous
"""Bass/Trainium2 kernel for nn_BipartiteSoftMatching (8 cores, batch-parallel).

Since r = t//2 the argsort in the reference is irrelevant: src_idx is a full
permutation and unm_idx is empty.  Per batch element the computation reduces to
  m = metric / ||metric||;  scores = m_even @ m_odd^T
  node_idx[i] = argmax_j scores[i, j]
  dst_out[j]  = (x_odd[j] + sum_{i: node_idx[i]=j} x_even[i]) / (1 + count[j])
  out[2j+1]   = dst_out[j];   out[2i] = dst_out[node_idx[i]]

Hardware mapping:
  - scores: fp32 matmuls, K=64 pairs row-packed via tile_position (exact
    enough; f32r would round operands to ~12 bits and flip argmaxes --
    measured min top-2 gap on this data is 2.7e-6).
  - argmax: DVE max (top-8) + max_index.
  - scatter-add, bucketed: tokens are permuted on-device into 16 buckets by
    destination j-tile (256 slots each, OOB-padded).  The permutation
    (slot = 256*bucket + cross-tile-count + within-tile-rank) is built with a
    transpose + lower-triangular-mask matmul trick, scattered to a DRAM
    staging table, and x_even rows are gathered back bucket-contiguous.
    Each j-tile then needs only 2 one-hot f32r matmuls per chunk/pass
    (vs 16) against x split into hi+lo f32r parts (2 passes ~ 2^-24 exact).
    A ones-column appended to x_hi yields the counts for free.
  - out even rows: per-tile indirect gathers of out's odd rows by
    2*node_idx+1, then one merged store.
"""

import numpy as np

import concourse.bacc as bacc
import concourse.bass as bass
import concourse.mybir as mybir
import concourse.tile as tile
from concourse.bass import IndirectOffsetOnAxis
from concourse.bass_utils import run_bass_kernel_spmd
from concourse.masks import make_identity

F32 = mybir.dt.float32
F32R = mybir.dt.float32r
U32 = mybir.dt.uint32
I32 = mybir.dt.int32
OP = mybir.AluOpType
AF = mybir.ActivationFunctionType

N, T, CM, CX = 8, 4096, 64, 768
P = 128
T1 = T // 2          # 2048 tokens per side
TI = T1 // P         # 16 i-tiles (even side)
TJ = T1 // P         # 16 j-tiles (odd side)
CXP = CX + 4         # x width padded: 768 data + 1 count col + 3 pad
NSLOT = 2 * T1       # 16 buckets x 256 slots
BIG = 1 << 20        # pad marker in the permutation table

_CACHE = {}


def _build(debug=False):
    nc = bacc.Bacc("TRN2", target_bir_lowering=False, num_devices=N)
    metric_in = nc.declare_dram_parameter("metric", [T, CM], F32, isOutput=False)
    x_in = nc.declare_dram_parameter("x", [T, CX], F32, isOutput=False)
    out = nc.declare_dram_parameter("out", [T, CX], F32, isOutput=True)
    if debug:
        perm8 = nc.declare_dram_parameter("perm8", [NSLOT, 2], I32, isOutput=True)
    else:
        perm8 = nc.dram_tensor("perm8", [NSLOT, 2], I32)

    # token = (t*128 + p)*2 + e
    m_pv = metric_in[:].rearrange("(t p e) c -> e p t c", p=P, e=2)
    x_r = x_in[:].rearrange("(t p e) c -> e t p c", p=P, e=2)
    out_r = out[:].rearrange("(t p e) c -> e t p c", p=P, e=2)
    out_pv = out[:].rearrange("(t p e) c -> e p t c", p=P, e=2)
    perm_pv = perm8[:].rearrange("(u p) w -> p u w", p=P)

    with tile.TileContext(nc, num_cores=N) as tc:
        with tc.tile_pool(name="const", bufs=1) as cp:
            ident = cp.tile([P, P], F32)
            make_identity(nc, ident[:])
            iota_row = cp.tile([P, T1], F32)
            nc.gpsimd.iota(iota_row[:], pattern=[[1, T1]], base=0,
                           channel_multiplier=0,
                           allow_small_or_imprecise_dtypes=True)
            iota16 = cp.tile([P, 16], F32)
            nc.gpsimd.iota(iota16[:], pattern=[[1, 16]], base=0,
                           channel_multiplier=0,
                           allow_small_or_imprecise_dtypes=True)
            ones128 = cp.tile([P, P], F32)
            nc.vector.memset(ones128[:], 1.0)
            # LT[p, f] = 1.0 if f < p else 0.0 (strict lower triangle)
            lt_i = cp.tile([P, P], I32)
            nc.gpsimd.iota(lt_i[:], pattern=[[1, P]], base=0,
                           channel_multiplier=-1)
            ltm = cp.tile([P, P], F32)
            nc.vector.tensor_scalar(ltm[:], lt_i[:], 0, None, op0=OP.is_lt)
            # xrow[p, t] = 2p + 256t = DRAM row of even token (t*128+p)
            xrow_i32 = cp.tile([P, TI], I32)
            nc.gpsimd.iota(xrow_i32[:], pattern=[[256, TI]], base=0,
                           channel_multiplier=2)
            cnt_pat = cp.tile([P, 4], F32)
            nc.vector.memset(cnt_pat[:], 0.0)
            nc.vector.memset(cnt_pat[:, 0:1], 1.0)
            bigpat = cp.tile([P, 2 * NSLOT // P], I32)
            nc.vector.memset(bigpat[:], BIG)

            idxf = cp.tile([P, TI], F32)
            offs = cp.tile([P, TI], I32)
            mi_all = cp.tile([P, TI * 8], U32)
            slot_f = cp.tile([P, TI], F32)
            slot_i32 = cp.tile([P, TI], I32)
            pr_all = cp.tile([P, TI * 2], I32)
            crun = cp.tile([1, 16], F32)       # running bucket counts
            nc.vector.memset(crun[:], 0.0)

            # pre-fill the permutation table with the OOB marker
            nc.sync.dma_start(out=perm8[:].rearrange("(p u) w -> p (u w)", p=P),
                              in_=bigpat[:])

            with tc.tile_pool(name="work", bufs=1) as wp:
                aTpk = wp.tile([P, T1 // 2], F32)
                bTpk = wp.tile([P, T1], F32)
                me = wp.tile([P, TI * CM], F32)
                mo = wp.tile([P, TI * CM], F32)

                nc.sync.dma_start(out=me[:].rearrange("p (t c) -> p t c", c=CM),
                                  in_=m_pv[0])
                nc.sync.dma_start(out=mo[:].rearrange("p (t c) -> p t c", c=CM),
                                  in_=m_pv[1])

                with tc.tile_pool(name="pA", bufs=3) as pa, \
                     tc.tile_pool(name="pB", bufs=2) as pb, \
                     tc.tile_pool(name="psA", bufs=2, space="PSUM") as psa, \
                     tc.tile_pool(name="psB", bufs=2, space="PSUM") as psb, \
                     tc.tile_pool(name="psR", bufs=1, space="PSUM") as psr:

                    # ---- Phase A: normalize metric, transpose directly into
                    # the packed operands (col-offset tile_position writes the
                    # upper partition half) ----
                    def normalize(src, t):
                        mt = src[:, t * CM:(t + 1) * CM]
                        sq = pa.tile([P, CM], F32, tag="sq")
                        ssum = pa.tile([P, 1], F32, tag="ss")
                        nc.scalar.activation(sq[:], mt, AF.Square,
                                             accum_out=ssum[:])
                        nrm = pa.tile([P, 1], F32, tag="nr")
                        nc.scalar.sqrt(nrm[:], ssum[:])
                        rnm = pa.tile([P, 1], F32, tag="rn")
                        nc.vector.reciprocal(rnm[:], nrm[:])
                        nm = pa.tile([P, CM], F32, tag="nm")
                        nc.vector.tensor_scalar_mul(nm[:], mt, rnm[:, 0:1])
                        return nm

                    for t in range(TI):          # odd side -> both bTpk halves
                        nm = normalize(mo, t)
                        pst = psa.tile([CM, P], F32, tag="tp", space="PSUM")
                        nc.tensor.transpose(pst[:], nm[:], ident[:])
                        blk = bTpk[:, t * P:(t + 1) * P]
                        nc.scalar.copy(blk[0:CM, :], pst[:])
                        nc.sync.dma_start(out=blk[CM:P, :], in_=blk[0:CM, :])
                    for t in range(TI):          # even side -> aTpk half by parity
                        nm = normalize(me, t)
                        pst = psa.tile([CM, P], F32, tag="tp", space="PSUM")
                        nc.tensor.transpose(pst[:], nm[:], ident[:])
                        blk = aTpk[:, (t // 2) * P:(t // 2 + 1) * P]
                        if t % 2 == 0:
                            nc.scalar.copy(blk[0:CM, :], pst[:])
                        else:
                            stg = pa.tile([CM, P], F32, tag="stg")
                            nc.scalar.copy(stg[:], pst[:])
                            nc.sync.dma_start(out=blk[CM:P, :], in_=stg[:])

                    # ---- Phase B: scores + argmax + slot computation ----
                    def rank_and_scatter(i):
                        """Per-tile permutation work: bucket, within-tile rank,
                        cross-tile count, slot, scatter (row, idx) to perm8."""
                        mi8 = mi_all[:, 8 * i:8 * i + 8]
                        nc.vector.tensor_copy(idxf[:, i:i + 1], mi8[:, 0:1])
                        nc.vector.tensor_copy(pr_all[:, 2 * i + 1:2 * i + 2],
                                              mi8[:, 0:1])
                        bu = pb.tile([P, 1], U32, tag="bu")
                        nc.vector.tensor_scalar(bu[:], mi8[:, 0:1], 7, None,
                                                op0=OP.logical_shift_right)
                        bf = pb.tile([P, 1], F32, tag="bf")
                        nc.gpsimd.tensor_copy(bf[:], bu[:])
                        # one-hot over 16 buckets
                        oh = pb.tile([P, 16], F32, tag="oh")
                        nc.vector.tensor_scalar(oh[:], iota16[:], bf[:, 0:1],
                                                None, op0=OP.is_equal)
                        # within-tile rank: W[p] = #{p' < p: b[p'] == b[p]}
                        bt_ps = psr.tile([P, P], F32, tag="bt", space="PSUM")
                        nc.tensor.transpose(bt_ps[:], bf[:].to_broadcast([P, P]),
                                            ident[:])
                        bt_sb = pb.tile([P, P], F32, tag="bts")
                        nc.scalar.copy(bt_sb[:], bt_ps[:])
                        scr = pb.tile([P, P], F32, tag="scr")
                        wcol = pb.tile([P, 1], F32, tag="wc")
                        nc.vector.scalar_tensor_tensor(
                            out=scr[:], in0=bt_sb[:], scalar=bf[:, 0:1],
                            in1=ltm[:], op0=OP.is_equal, op1=OP.mult,
                            accum_out=wcol[:])
                        # cross-tile count of this bucket so far
                        cb = pb.tile([P, 16], F32, tag="cb")
                        nc.gpsimd.partition_broadcast(cb[:], crun[:])
                        ctv = pb.tile([P, 1], F32, tag="ctv")
                        s2 = pb.tile([P, 16], F32, tag="s2")
                        nc.vector.scalar_tensor_tensor(
                            out=s2[:], in0=oh[:], scalar=1.0, in1=cb[:],
                            op0=OP.mult, op1=OP.mult, accum_out=ctv[:])
                        # update running counts: crun += hist(b) (via matmul)
                        hp = psr.tile([1, 16], F32, tag="hp", space="PSUM")
                        nc.tensor.matmul(hp[:], ones128[:, 0:1], oh[:],
                                         start=True, stop=True)
                        nc.vector.tensor_add(crun[:], crun[:], hp[:])
                        # slot = 256*b + ctv + W
                        sf = slot_f[:, i:i + 1]
                        nc.vector.scalar_tensor_tensor(
                            out=sf, in0=bf[:], scalar=256.0, in1=ctv[:],
                            op0=OP.mult, op1=OP.add)
                        nc.vector.tensor_tensor(out=sf, in0=sf, in1=wcol[:],
                                                op=OP.add)
                        nc.vector.tensor_copy(slot_i32[:, i:i + 1], sf)
                        nc.vector.tensor_copy(pr_all[:, 2 * i:2 * i + 1],
                                              xrow_i32[:, i:i + 1])
                        nc.gpsimd.indirect_dma_start(
                            out=perm8[:], in_=pr_all[:, 2 * i:2 * i + 2],
                            in_offset=None,
                            out_offset=IndirectOffsetOnAxis(
                                ap=slot_i32[:, i:i + 1], axis=0))

                    for ii in range(TI // 2):
                        i0, i1 = 2 * ii, 2 * ii + 1
                        ssb0 = pb.tile([P, T1], F32, tag="scores0")
                        ssb1 = pb.tile([P, T1], F32, tag="scores1")
                        for c in range(4):
                            nj = c * 512
                            ps0 = psb.tile([P, 512], F32, tag="ps0", space="PSUM")
                            ps1 = psb.tile([P, 512], F32, tag="ps1", space="PSUM")
                            nc.tensor.matmul(ps0[:], aTpk[0:CM, ii * P:(ii + 1) * P],
                                             bTpk[0:CM, nj:nj + 512],
                                             start=True, stop=True,
                                             tile_position=(0, 0))
                            nc.tensor.matmul(ps1[:], aTpk[CM:P, ii * P:(ii + 1) * P],
                                             bTpk[CM:P, nj:nj + 512],
                                             start=True, stop=True,
                                             tile_position=(64, 0))
                            nc.scalar.copy(ssb0[:, nj:nj + 512], ps0[:])
                            nc.scalar.copy(ssb1[:, nj:nj + 512], ps1[:])
                        for i, ssb in ((i0, ssb0), (i1, ssb1)):
                            mx8 = pb.tile([P, 8], F32, tag="mx")
                            nc.vector.max(out=mx8[:], in_=ssb[:])
                            nc.vector.max_index(out=mi_all[:, 8 * i:8 * i + 8],
                                                in_max=mx8[:], in_values=ssb[:])
                            rank_and_scatter(i)

                    # gather offsets for phase D (odd row of out = 2*idx+1)
                    off_f = pb.tile([P, TI], F32)
                    nc.vector.tensor_scalar(off_f[:], idxf[:], 2.0, 1.0,
                                            op0=OP.mult, op1=OP.add)
                    nc.vector.tensor_copy(offs[:], off_f[:])

            # ---- Phase C: bucketed one-hot scatter matmul ----
            with tc.tile_pool(name="pq", bufs=1) as pqp, \
                 tc.tile_pool(name="pC", bufs=6) as pcs, \
                 tc.tile_pool(name="pD", bufs=2) as pd, \
                 tc.tile_pool(name="psC", bufs=2, space="PSUM") as psc:
                pq = pqp.tile([P, NSLOT // P * 2], I32)
                nc.sync.dma_start(
                    out=pq[:].rearrange("p (u w) -> p u w", w=2), in_=perm_pv)
                pq_v = pq[:].rearrange("p (u w) -> p u w", w=2)
                idxg_f = pqp.tile([P, NSLOT // P], F32)
                nc.vector.tensor_copy(idxg_f[:], pq_v[:, :, 1])
                # clamp pad-slot offsets (BIG) to a valid row; their one-hot
                # row is all-zero so the gathered data is multiplied away
                qoff = pqp.tile([P, NSLOT // P], I32)
                nc.vector.tensor_scalar(qoff[:], pq_v[:, :, 0], T - 2, None,
                                        op0=OP.min)

                ones_r = pqp.tile([P, 2], F32R)
                nc.vector.tensor_copy(ones_r[:], ones128[:, 0:2])
                for jt in range(TJ):
                    psj = psc.tile([P, CX], F32, tag="sp", space="PSUM")
                    psn = psc.tile([P, 2], F32, tag="sn", space="PSUM")
                    for k in range(2):
                        u = 2 * jt + k
                        xg = pcs.tile([P, CX], F32, tag="xg")
                        nc.gpsimd.indirect_dma_start(
                            out=xg[:], out_offset=None,
                            in_=x_in[:],
                            in_offset=IndirectOffsetOnAxis(
                                ap=qoff[:, u:u + 1], axis=0))
                        xh = pcs.tile([P, CX], F32R, tag="xh")
                        xl = pcs.tile([P, CX], F32R, tag="xl")
                        nc.vector.tensor_copy(xh[:], xg[:])
                        sub_eng = nc.vector if u % 2 == 0 else nc.gpsimd
                        sub_eng.tensor_tensor(out=xl[:], in0=xg[:],
                                              in1=xh[:].bitcast(F32),
                                              op=OP.subtract)
                        eqr = pcs.tile([P, P], F32R, tag="eq")
                        nc.vector.scalar_tensor_tensor(
                            out=eqr[:],
                            in0=iota_row[:, jt * P:(jt + 1) * P],
                            scalar=idxg_f[:, u:u + 1],
                            in1=ones128[:],
                            op0=OP.is_equal, op1=OP.mult)
                        first, last = (k == 0), (k == 1)
                        nc.tensor.matmul(psn[:], eqr[:], ones_r[:],
                                         start=first, stop=last)
                        for lo_, hi_ in ((0, 512), (512, CX)):
                            nc.tensor.matmul(psj[:, lo_:hi_], eqr[:],
                                             xh[:, lo_:hi_],
                                             start=first, stop=False)
                            nc.tensor.matmul(psj[:, lo_:hi_], eqr[:],
                                             xl[:, lo_:hi_],
                                             start=False, stop=last)
                    xo = pd.tile([P, CX], F32, tag="xo")
                    nc.sync.dma_start(out=xo[:], in_=x_r[1, jt])
                    cnt1 = pd.tile([P, 1], F32, tag="c1")
                    nc.vector.tensor_scalar_add(cnt1[:], psn[:, 0:1], 1.0)
                    inv = pd.tile([P, 1], F32, tag="iv")
                    nc.vector.reciprocal(inv[:], cnt1[:])
                    dst = pd.tile([P, CX], F32, tag="dst")
                    nc.vector.tensor_add(dst[:], xo[:], psj[:, 0:CX])
                    nc.scalar.mul(dst[:], dst[:], inv[:, 0:1])
                    nc.sync.dma_start(out=out_r[1, jt], in_=dst[:])

            # ---- Phase D: gather even rows, store in 4-tile chunks ----
            with tc.tile_pool(name="pG", bufs=1) as pg:
                for q in range(TI // 4):
                    gb = pg.tile([P, 4 * CX], F32, name=f"gb{q}", tag=f"gb{q}")
                    for k in range(4):
                        i = 4 * q + k
                        nc.gpsimd.indirect_dma_start(
                            out=gb[:, k * CX:(k + 1) * CX], out_offset=None,
                            in_=out[:],
                            in_offset=IndirectOffsetOnAxis(ap=offs[:, i:i + 1],
                                                           axis=0))
                    nc.sync.dma_start(
                        out=out_pv[0][:, 4 * q:4 * q + 4, :],
                        in_=gb[:].rearrange("p (t c) -> p t c", c=CX))

    nc.compile()
    return nc


def kernel(metric: np.ndarray, x: np.ndarray) -> np.ndarray:
    if "nc" not in _CACHE:
        _CACHE["nc"] = _build()
    nc = _CACHE["nc"]
    metric = np.ascontiguousarray(np.asarray(metric, dtype=np.float32))
    x = np.ascontiguousarray(np.asarray(x, dtype=np.float32))
    in_maps = [{"metric": metric[c], "x": x[c]} for c in range(N)]
    res = run_bass_kernel_spmd(nc, in_maps, list(range(N)))
    return np.stack([res.results[c]["out"] for c in range(N)], axis=0)


# revision 41
# speedup vs baseline: 1.1251x; 1.1077x over previous
"""Bass/Trainium2 kernel for nn_BipartiteSoftMatching (8 cores, batch-parallel).

Since r = t//2 the argsort in the reference is irrelevant: src_idx is a full
permutation and unm_idx is empty.  Per batch element the computation reduces to
  m = metric / ||metric||;  scores = m_even @ m_odd^T
  node_idx[i] = argmax_j scores[i, j]
  dst_out[j]  = (x_odd[j] + sum_{i: node_idx[i]=j} x_even[i]) / (1 + count[j])
  out[2j+1]   = dst_out[j];   out[2i] = dst_out[node_idx[i]]

Hardware mapping:
  - scores: fp32 matmuls, K=64 pairs row-packed via tile_position (exact
    enough; f32r would round operands to ~12 bits and flip argmaxes --
    measured min top-2 gap on this data is 2.7e-6).
  - argmax: DVE max (top-8) + max_index.
  - scatter-add, bucketed: tokens are permuted on-device into 16 buckets by
    destination j-tile (256 slots each, OOB-padded).  The permutation
    (slot = 256*bucket + cross-tile-count + within-tile-rank) is built with a
    transpose + lower-triangular-mask matmul trick, scattered to a DRAM
    staging table, and x_even rows are gathered back bucket-contiguous.
    Each j-tile then needs only 2 one-hot f32r matmuls per chunk/pass
    (vs 16) against x split into hi+lo f32r parts (2 passes ~ 2^-24 exact).
    A ones-column appended to x_hi yields the counts for free.
  - out even rows: per-tile indirect gathers of out's odd rows by
    2*node_idx+1, then one merged store.
"""

import numpy as np

import concourse.bacc as bacc
import concourse.bass as bass
import concourse.mybir as mybir
import concourse.tile as tile
from concourse.bass import IndirectOffsetOnAxis
from concourse.bass_utils import run_bass_kernel_spmd
from concourse.masks import make_identity

F32 = mybir.dt.float32
F32R = mybir.dt.float32r
U32 = mybir.dt.uint32
I32 = mybir.dt.int32
OP = mybir.AluOpType
AF = mybir.ActivationFunctionType

N, T, CM, CX = 8, 4096, 64, 768
P = 128
T1 = T // 2          # 2048 tokens per side
TI = T1 // P         # 16 i-tiles (even side)
TJ = T1 // P         # 16 j-tiles (odd side)
CXP = CX + 4         # x width padded: 768 data + 1 count col + 3 pad
NSLOT = 2 * T1       # 16 buckets x 256 slots
BIG = 1 << 20        # pad marker in the permutation table

_CACHE = {}


def _build(debug=False):
    nc = bacc.Bacc("TRN2", target_bir_lowering=False, num_devices=N)
    metric_in = nc.declare_dram_parameter("metric", [T, CM], F32, isOutput=False)
    x_in = nc.declare_dram_parameter("x", [T, CX], F32, isOutput=False)
    out = nc.declare_dram_parameter("out", [T, CX], F32, isOutput=True)
    if debug:
        perm8 = nc.declare_dram_parameter("perm8", [NSLOT, 2], I32, isOutput=True)
    else:
        perm8 = nc.dram_tensor("perm8", [NSLOT, 2], I32)

    # token = (t*128 + p)*2 + e
    m_pv = metric_in[:].rearrange("(t p e) c -> e p t c", p=P, e=2)
    x_r = x_in[:].rearrange("(t p e) c -> e t p c", p=P, e=2)
    out_r = out[:].rearrange("(t p e) c -> e t p c", p=P, e=2)
    out_pv = out[:].rearrange("(t p e) c -> e p t c", p=P, e=2)
    perm_pv = perm8[:].rearrange("(u p) w -> p u w", p=P)

    with tile.TileContext(nc, num_cores=N) as tc:
        with tc.tile_pool(name="const", bufs=1) as cp:
            ident = cp.tile([P, P], F32)
            make_identity(nc, ident[:])
            iota_row = cp.tile([P, T1], F32)
            nc.gpsimd.iota(iota_row[:], pattern=[[1, T1]], base=0,
                           channel_multiplier=0,
                           allow_small_or_imprecise_dtypes=True)
            iota16 = cp.tile([P, 16], F32)
            nc.gpsimd.iota(iota16[:], pattern=[[1, 16]], base=0,
                           channel_multiplier=0,
                           allow_small_or_imprecise_dtypes=True)
            ones128 = cp.tile([P, P], F32)
            nc.vector.memset(ones128[:], 1.0)
            # LT[p, f] = 1.0 if f < p else 0.0 (strict lower triangle)
            lt_i = cp.tile([P, P], I32)
            nc.gpsimd.iota(lt_i[:], pattern=[[1, P]], base=0,
                           channel_multiplier=-1)
            ltm = cp.tile([P, P], F32)
            nc.vector.tensor_scalar(ltm[:], lt_i[:], 0, None, op0=OP.is_lt)
            # xrow[p, t] = 2p + 256t = DRAM row of even token (t*128+p)
            xrow_i32 = cp.tile([P, TI], I32)
            nc.gpsimd.iota(xrow_i32[:], pattern=[[256, TI]], base=0,
                           channel_multiplier=2)
            cnt_pat = cp.tile([P, 4], F32)
            nc.vector.memset(cnt_pat[:], 0.0)
            nc.vector.memset(cnt_pat[:, 0:1], 1.0)
            bigpat = cp.tile([P, 2 * NSLOT // P], I32)
            nc.vector.memset(bigpat[:], BIG)

            idxf = cp.tile([P, TI], F32)
            offs = cp.tile([P, TI], I32)
            mi_all = cp.tile([P, TI * 8], U32)
            slot_f = cp.tile([P, TI], F32)
            slot_i32 = cp.tile([P, TI], I32)
            pr_all = cp.tile([P, TI * 2], I32)
            crun = cp.tile([1, 16], F32)       # running bucket counts
            nc.vector.memset(crun[:], 0.0)

            # pre-fill the permutation table with the OOB marker
            nc.sync.dma_start(out=perm8[:].rearrange("(p u) w -> p (u w)", p=P),
                              in_=bigpat[:])

            with tc.tile_pool(name="work", bufs=1) as wp:
                aTpk = wp.tile([P, T1 // 2], F32)
                bTpk = wp.tile([P, T1], F32)
                me = wp.tile([P, TI * CM], F32)
                mo = wp.tile([P, TI * CM], F32)

                nc.sync.dma_start(out=me[:].rearrange("p (t c) -> p t c", c=CM),
                                  in_=m_pv[0])
                nc.sync.dma_start(out=mo[:].rearrange("p (t c) -> p t c", c=CM),
                                  in_=m_pv[1])

                with tc.tile_pool(name="pA", bufs=3) as pa, \
                     tc.tile_pool(name="pB", bufs=2) as pb, \
                     tc.tile_pool(name="psA", bufs=2, space="PSUM") as psa, \
                     tc.tile_pool(name="psB", bufs=2, space="PSUM") as psb, \
                     tc.tile_pool(name="psR", bufs=1, space="PSUM") as psr:

                    # ---- Phase A: normalize metric, transpose directly into
                    # the packed operands (col-offset tile_position writes the
                    # upper partition half) ----
                    def normalize(src, t):
                        mt = src[:, t * CM:(t + 1) * CM]
                        sq = pa.tile([P, CM], F32, tag="sq")
                        ssum = pa.tile([P, 1], F32, tag="ss")
                        nc.scalar.activation(sq[:], mt, AF.Square,
                                             accum_out=ssum[:])
                        nrm = pa.tile([P, 1], F32, tag="nr")
                        nc.scalar.sqrt(nrm[:], ssum[:])
                        rnm = pa.tile([P, 1], F32, tag="rn")
                        nc.vector.reciprocal(rnm[:], nrm[:])
                        nm = pa.tile([P, CM], F32, tag="nm")
                        nc.vector.tensor_scalar_mul(nm[:], mt, rnm[:, 0:1])
                        return nm

                    for t in range(TI):          # odd side -> both bTpk halves
                        nm = normalize(mo, t)
                        pst = psa.tile([CM, P], F32, tag="tp", space="PSUM")
                        nc.tensor.transpose(pst[:], nm[:], ident[:])
                        blk = bTpk[:, t * P:(t + 1) * P]
                        nc.scalar.copy(blk[0:CM, :], pst[:])
                        nc.sync.dma_start(out=blk[CM:P, :], in_=blk[0:CM, :])
                    for t in range(TI):          # even side -> aTpk half by parity
                        nm = normalize(me, t)
                        pst = psa.tile([CM, P], F32, tag="tp", space="PSUM")
                        nc.tensor.transpose(pst[:], nm[:], ident[:])
                        blk = aTpk[:, (t // 2) * P:(t // 2 + 1) * P]
                        if t % 2 == 0:
                            nc.scalar.copy(blk[0:CM, :], pst[:])
                        else:
                            stg = pa.tile([CM, P], F32, tag="stg")
                            nc.scalar.copy(stg[:], pst[:])
                            nc.sync.dma_start(out=blk[CM:P, :], in_=stg[:])

                    # ---- Phase B: scores + argmax + slot computation ----
                    def rank_and_scatter(i):
                        """Per-tile permutation work: bucket, within-tile rank,
                        cross-tile count, slot, scatter (row, idx) to perm8."""
                        mi8 = mi_all[:, 8 * i:8 * i + 8]
                        nc.vector.tensor_copy(idxf[:, i:i + 1], mi8[:, 0:1])
                        nc.vector.tensor_copy(pr_all[:, 2 * i + 1:2 * i + 2],
                                              mi8[:, 0:1])
                        bu = pb.tile([P, 1], U32, tag="bu")
                        nc.vector.tensor_scalar(bu[:], mi8[:, 0:1], 7, None,
                                                op0=OP.logical_shift_right)
                        bf = pb.tile([P, 1], F32, tag="bf")
                        nc.gpsimd.tensor_copy(bf[:], bu[:])
                        # one-hot over 16 buckets
                        oh = pb.tile([P, 16], F32, tag="oh")
                        nc.vector.tensor_scalar(oh[:], iota16[:], bf[:, 0:1],
                                                None, op0=OP.is_equal)
                        # within-tile rank: W[p] = #{p' < p: b[p'] == b[p]}
                        bt_ps = psr.tile([P, P], F32, tag="bt", space="PSUM")
                        nc.tensor.transpose(bt_ps[:], bf[:].to_broadcast([P, P]),
                                            ident[:])
                        bt_sb = pb.tile([P, P], F32, tag="bts")
                        nc.scalar.copy(bt_sb[:], bt_ps[:])
                        scr = pb.tile([P, P], F32, tag="scr")
                        wcol = pb.tile([P, 1], F32, tag="wc")
                        nc.vector.scalar_tensor_tensor(
                            out=scr[:], in0=bt_sb[:], scalar=bf[:, 0:1],
                            in1=ltm[:], op0=OP.is_equal, op1=OP.mult,
                            accum_out=wcol[:])
                        # cross-tile count of this bucket so far
                        cb = pb.tile([P, 16], F32, tag="cb")
                        nc.gpsimd.partition_broadcast(cb[:], crun[:])
                        ctv = pb.tile([P, 1], F32, tag="ctv")
                        s2 = pb.tile([P, 16], F32, tag="s2")
                        nc.vector.scalar_tensor_tensor(
                            out=s2[:], in0=oh[:], scalar=1.0, in1=cb[:],
                            op0=OP.mult, op1=OP.mult, accum_out=ctv[:])
                        # update running counts: crun += hist(b) (via matmul)
                        hp = psr.tile([1, 16], F32, tag="hp", space="PSUM")
                        nc.tensor.matmul(hp[:], ones128[:, 0:1], oh[:],
                                         start=True, stop=True)
                        nc.vector.tensor_add(crun[:], crun[:], hp[:])
                        # slot = 256*b + ctv + W
                        sf = slot_f[:, i:i + 1]
                        nc.vector.scalar_tensor_tensor(
                            out=sf, in0=bf[:], scalar=256.0, in1=ctv[:],
                            op0=OP.mult, op1=OP.add)
                        nc.vector.tensor_tensor(out=sf, in0=sf, in1=wcol[:],
                                                op=OP.add)
                        nc.vector.tensor_copy(slot_i32[:, i:i + 1], sf)
                        nc.vector.tensor_copy(pr_all[:, 2 * i:2 * i + 1],
                                              xrow_i32[:, i:i + 1])
                        nc.gpsimd.indirect_dma_start(
                            out=perm8[:], in_=pr_all[:, 2 * i:2 * i + 2],
                            in_offset=None,
                            out_offset=IndirectOffsetOnAxis(
                                ap=slot_i32[:, i:i + 1], axis=0))

                    for ii in range(TI // 2):
                        i0, i1 = 2 * ii, 2 * ii + 1
                        ssb0 = pb.tile([P, T1], F32, tag="scores0")
                        ssb1 = pb.tile([P, T1], F32, tag="scores1")
                        for c in range(4):
                            nj = c * 512
                            ps0 = psb.tile([P, 512], F32, tag="ps0", space="PSUM")
                            ps1 = psb.tile([P, 512], F32, tag="ps1", space="PSUM")
                            nc.tensor.matmul(ps0[:], aTpk[0:CM, ii * P:(ii + 1) * P],
                                             bTpk[0:CM, nj:nj + 512],
                                             start=True, stop=True,
                                             tile_position=(0, 0))
                            nc.tensor.matmul(ps1[:], aTpk[CM:P, ii * P:(ii + 1) * P],
                                             bTpk[CM:P, nj:nj + 512],
                                             start=True, stop=True,
                                             tile_position=(64, 0))
                            nc.scalar.copy(ssb0[:, nj:nj + 512], ps0[:])
                            nc.scalar.copy(ssb1[:, nj:nj + 512], ps1[:])
                        for i, ssb in ((i0, ssb0), (i1, ssb1)):
                            mx8 = pb.tile([P, 8], F32, tag="mx")
                            nc.vector.max(out=mx8[:], in_=ssb[:])
                            nc.vector.max_index(out=mi_all[:, 8 * i:8 * i + 8],
                                                in_max=mx8[:], in_values=ssb[:])
                            rank_and_scatter(i)

                    # gather offsets for phase D (odd row of out = 2*idx+1)
                    off_f = pb.tile([P, TI], F32)
                    nc.vector.tensor_scalar(off_f[:], idxf[:], 2.0, 1.0,
                                            op0=OP.mult, op1=OP.add)
                    nc.vector.tensor_copy(offs[:], off_f[:])

            # ---- Phase C: bucketed one-hot scatter matmul ----
            with tc.tile_pool(name="pq", bufs=1) as pqp, \
                 tc.tile_pool(name="pC", bufs=6) as pcs, \
                 tc.tile_pool(name="pD", bufs=2) as pd, \
                 tc.tile_pool(name="psC", bufs=2, space="PSUM") as psc:
                pq = pqp.tile([P, NSLOT // P * 2], I32)
                nc.sync.dma_start(
                    out=pq[:].rearrange("p (u w) -> p u w", w=2), in_=perm_pv)
                pq_v = pq[:].rearrange("p (u w) -> p u w", w=2)
                idxg_f = pqp.tile([P, NSLOT // P], F32)
                nc.vector.tensor_copy(idxg_f[:], pq_v[:, :, 1])
                # clamp pad-slot offsets (BIG) to a valid row; their one-hot
                # row is all-zero so the gathered data is multiplied away
                qoff = pqp.tile([P, NSLOT // P], I32)
                nc.vector.tensor_scalar(qoff[:], pq_v[:, :, 0], T - 2, None,
                                        op0=OP.min)

                ones_r = pqp.tile([P, 2], F32R)
                nc.vector.tensor_copy(ones_r[:], ones128[:, 0:2])
                for jt in range(TJ):
                    psj = psc.tile([P, CX], F32, tag="sp", space="PSUM")
                    psn = psc.tile([P, 2], F32, tag="sn", space="PSUM")
                    for k in range(2):
                        u = 2 * jt + k
                        xg = pcs.tile([P, CX], F32, tag="xg")
                        nc.gpsimd.indirect_dma_start(
                            out=xg[:], out_offset=None,
                            in_=x_in[:],
                            in_offset=IndirectOffsetOnAxis(
                                ap=qoff[:, u:u + 1], axis=0))
                        xh = pcs.tile([P, CX], F32R, tag="xh")
                        xl = pcs.tile([P, CX], F32R, tag="xl")
                        nc.vector.tensor_copy(xh[:], xg[:])
                        nc.vector.tensor_tensor(out=xl[:], in0=xg[:],
                                                in1=xh[:].bitcast(F32),
                                                op=OP.subtract)
                        eqr = pcs.tile([P, P], F32R, tag="eq")
                        nc.vector.scalar_tensor_tensor(
                            out=eqr[:],
                            in0=iota_row[:, jt * P:(jt + 1) * P],
                            scalar=idxg_f[:, u:u + 1],
                            in1=ones128[:],
                            op0=OP.is_equal, op1=OP.mult)
                        first, last = (k == 0), (k == 1)
                        nc.tensor.matmul(psn[:], eqr[:], ones_r[:],
                                         start=first, stop=last)
                        for lo_, hi_ in ((0, 512), (512, CX)):
                            nc.tensor.matmul(psj[:, lo_:hi_], eqr[:],
                                             xh[:, lo_:hi_],
                                             start=first, stop=False)
                            nc.tensor.matmul(psj[:, lo_:hi_], eqr[:],
                                             xl[:, lo_:hi_],
                                             start=False, stop=last)
                    xo = pd.tile([P, CX], F32, tag="xo")
                    nc.sync.dma_start(out=xo[:], in_=x_r[1, jt])
                    cnt1 = pd.tile([P, 1], F32, tag="c1")
                    nc.vector.tensor_scalar_add(cnt1[:], psn[:, 0:1], 1.0)
                    inv = pd.tile([P, 1], F32, tag="iv")
                    nc.vector.reciprocal(inv[:], cnt1[:])
                    dst = pd.tile([P, CX], F32, tag="dst")
                    nc.vector.tensor_add(dst[:], xo[:], psj[:, 0:CX])
                    nc.scalar.mul(dst[:], dst[:], inv[:, 0:1])
                    nc.sync.dma_start(out=out_r[1, jt], in_=dst[:])

            # ---- Phase D: gather even rows, store in 4-tile chunks ----
            with tc.tile_pool(name="pG", bufs=1) as pg:
                for q in range(TI // 4):
                    gb = pg.tile([P, 4 * CX], F32, name=f"gb{q}", tag=f"gb{q}")
                    for k in range(4):
                        i = 4 * q + k
                        nc.gpsimd.indirect_dma_start(
                            out=gb[:, k * CX:(k + 1) * CX], out_offset=None,
                            in_=out[:],
                            in_offset=IndirectOffsetOnAxis(ap=offs[:, i:i + 1],
                                                           axis=0))
                    nc.sync.dma_start(
                        out=out_pv[0][:, 4 * q:4 * q + 4, :],
                        in_=gb[:].rearrange("p (t c) -> p t c", c=CX))

    nc.compile()
    return nc


def kernel(metric: np.ndarray, x: np.ndarray) -> np.ndarray:
    if "nc" not in _CACHE:
        _CACHE["nc"] = _build()
    nc = _CACHE["nc"]
    metric = np.ascontiguousarray(np.asarray(metric, dtype=np.float32))
    x = np.ascontiguousarray(np.asarray(x, dtype=np.float32))
    in_maps = [{"metric": metric[c], "x": x[c]} for c in range(N)]
    res = run_bass_kernel_spmd(nc, in_maps, list(range(N)))
    return np.stack([res.results[c]["out"] for c in range(N)], axis=0)


# revision 42
# speedup vs baseline: 1.1664x; 1.0367x over previous
"""Bass/Trainium2 kernel for nn_BipartiteSoftMatching (8 cores, batch-parallel).

Since r = t//2 the argsort in the reference is irrelevant: src_idx is a full
permutation and unm_idx is empty.  Per batch element the computation reduces to
  m = metric / ||metric||;  scores = m_even @ m_odd^T
  node_idx[i] = argmax_j scores[i, j]
  dst_out[j]  = (x_odd[j] + sum_{i: node_idx[i]=j} x_even[i]) / (1 + count[j])
  out[2j+1]   = dst_out[j];   out[2i] = dst_out[node_idx[i]]

Hardware mapping:
  - scores: fp32 matmuls, K=64 pairs row-packed via tile_position (exact
    enough; f32r would round operands to ~12 bits and flip argmaxes --
    measured min top-2 gap on this data is 2.7e-6).
  - argmax: DVE max (top-8) + max_index.
  - scatter-add, bucketed: tokens are permuted on-device into 16 buckets by
    destination j-tile (256 slots each, OOB-padded).  The permutation
    (slot = 256*bucket + cross-tile-count + within-tile-rank) is built with a
    transpose + lower-triangular-mask matmul trick, scattered to a DRAM
    staging table, and x_even rows are gathered back bucket-contiguous.
    Each j-tile then needs only 2 one-hot f32r matmuls per chunk/pass
    (vs 16) against x split into hi+lo f32r parts (2 passes ~ 2^-24 exact).
    A ones-column appended to x_hi yields the counts for free.
  - out even rows: per-tile indirect gathers of out's odd rows by
    2*node_idx+1, then one merged store.
"""

import numpy as np

import concourse.bacc as bacc
import concourse.bass as bass
import concourse.mybir as mybir
import concourse.tile as tile
from concourse.bass import IndirectOffsetOnAxis
from concourse.bass_utils import run_bass_kernel_spmd
from concourse.masks import make_identity

F32 = mybir.dt.float32
F32R = mybir.dt.float32r
U32 = mybir.dt.uint32
I32 = mybir.dt.int32
OP = mybir.AluOpType
AF = mybir.ActivationFunctionType

N, T, CM, CX = 8, 4096, 64, 768
P = 128
T1 = T // 2          # 2048 tokens per side
TI = T1 // P         # 16 i-tiles (even side)
TJ = T1 // P         # 16 j-tiles (odd side)
CXP = CX + 4         # x width padded: 768 data + 1 count col + 3 pad
NSLOT = 2 * T1       # 16 buckets x 256 slots
BIG = 1 << 20        # pad marker in the permutation table

_CACHE = {}


def _build(debug=False):
    nc = bacc.Bacc("TRN2", target_bir_lowering=False, num_devices=N)
    metric_in = nc.declare_dram_parameter("metric", [T, CM], F32, isOutput=False)
    x_in = nc.declare_dram_parameter("x", [T, CX], F32, isOutput=False)
    out = nc.declare_dram_parameter("out", [T, CX], F32, isOutput=True)
    if debug:
        perm8 = nc.declare_dram_parameter("perm8", [NSLOT, 2], I32, isOutput=True)
    else:
        perm8 = nc.dram_tensor("perm8", [NSLOT, 2], I32)
    dstbuf = nc.dram_tensor("dstbuf", [T1, CX], F32)

    # token = (t*128 + p)*2 + e
    m_pv = metric_in[:].rearrange("(t p e) c -> e p t c", p=P, e=2)
    x_r = x_in[:].rearrange("(t p e) c -> e t p c", p=P, e=2)
    out_r = out[:].rearrange("(t p e) c -> e t p c", p=P, e=2)
    out_pv = out[:].rearrange("(t p e) c -> e p t c", p=P, e=2)
    perm_pv = perm8[:].rearrange("(u p) w -> p u w", p=P)

    with tile.TileContext(nc, num_cores=N) as tc:
        with tc.tile_pool(name="const", bufs=1) as cp:
            ident = cp.tile([P, P], F32)
            make_identity(nc, ident[:])
            iota_row = cp.tile([P, T1], F32)
            nc.gpsimd.iota(iota_row[:], pattern=[[1, T1]], base=0,
                           channel_multiplier=0,
                           allow_small_or_imprecise_dtypes=True)
            iota16 = cp.tile([P, 16], F32)
            nc.gpsimd.iota(iota16[:], pattern=[[1, 16]], base=0,
                           channel_multiplier=0,
                           allow_small_or_imprecise_dtypes=True)
            ones128 = cp.tile([P, P], F32)
            nc.vector.memset(ones128[:], 1.0)
            # LT[p, f] = 1.0 if f < p else 0.0 (strict lower triangle)
            lt_i = cp.tile([P, P], I32)
            nc.gpsimd.iota(lt_i[:], pattern=[[1, P]], base=0,
                           channel_multiplier=-1)
            ltm = cp.tile([P, P], F32)
            nc.vector.tensor_scalar(ltm[:], lt_i[:], 0, None, op0=OP.is_lt)
            # xrow[p, t] = 2p + 256t = DRAM row of even token (t*128+p)
            xrow_i32 = cp.tile([P, TI], I32)
            nc.gpsimd.iota(xrow_i32[:], pattern=[[256, TI]], base=0,
                           channel_multiplier=2)
            cnt_pat = cp.tile([P, 4], F32)
            nc.vector.memset(cnt_pat[:], 0.0)
            nc.vector.memset(cnt_pat[:, 0:1], 1.0)
            bigpat = cp.tile([P, 2 * NSLOT // P], I32)
            nc.vector.memset(bigpat[:], BIG)

            idxf = cp.tile([P, TI], F32)
            offs = cp.tile([P, TI], I32)
            mi_all = cp.tile([P, TI * 8], U32)
            slot_f = cp.tile([P, TI], F32)
            slot_i32 = cp.tile([P, TI], I32)
            pr_all = cp.tile([P, TI * 2], I32)
            crun = cp.tile([1, 16], F32)       # running bucket counts
            nc.vector.memset(crun[:], 0.0)

            # pre-fill the permutation table with the OOB marker
            nc.sync.dma_start(out=perm8[:].rearrange("(p u) w -> p (u w)", p=P),
                              in_=bigpat[:])

            with tc.tile_pool(name="work", bufs=1) as wp:
                aTpk = wp.tile([P, T1 // 2], F32)
                bTpk = wp.tile([P, T1], F32)
                me = wp.tile([P, TI * CM], F32)
                mo = wp.tile([P, TI * CM], F32)

                nc.sync.dma_start(out=me[:].rearrange("p (t c) -> p t c", c=CM),
                                  in_=m_pv[0])
                nc.sync.dma_start(out=mo[:].rearrange("p (t c) -> p t c", c=CM),
                                  in_=m_pv[1])

                with tc.tile_pool(name="pA", bufs=3) as pa, \
                     tc.tile_pool(name="pB", bufs=2) as pb, \
                     tc.tile_pool(name="psA", bufs=2, space="PSUM") as psa, \
                     tc.tile_pool(name="psB", bufs=2, space="PSUM") as psb, \
                     tc.tile_pool(name="psR", bufs=1, space="PSUM") as psr:

                    # ---- Phase A: normalize metric, transpose directly into
                    # the packed operands (col-offset tile_position writes the
                    # upper partition half) ----
                    def normalize(src, t):
                        mt = src[:, t * CM:(t + 1) * CM]
                        sq = pa.tile([P, CM], F32, tag="sq")
                        ssum = pa.tile([P, 1], F32, tag="ss")
                        nc.scalar.activation(sq[:], mt, AF.Square,
                                             accum_out=ssum[:])
                        nrm = pa.tile([P, 1], F32, tag="nr")
                        nc.scalar.sqrt(nrm[:], ssum[:])
                        rnm = pa.tile([P, 1], F32, tag="rn")
                        nc.vector.reciprocal(rnm[:], nrm[:])
                        nm = pa.tile([P, CM], F32, tag="nm")
                        nc.vector.tensor_scalar_mul(nm[:], mt, rnm[:, 0:1])
                        return nm

                    for t in range(TI):          # odd side -> both bTpk halves
                        nm = normalize(mo, t)
                        pst = psa.tile([CM, P], F32, tag="tp", space="PSUM")
                        nc.tensor.transpose(pst[:], nm[:], ident[:])
                        blk = bTpk[:, t * P:(t + 1) * P]
                        nc.scalar.copy(blk[0:CM, :], pst[:])
                        nc.sync.dma_start(out=blk[CM:P, :], in_=blk[0:CM, :])
                    for t in range(TI):          # even side -> aTpk half by parity
                        nm = normalize(me, t)
                        pst = psa.tile([CM, P], F32, tag="tp", space="PSUM")
                        nc.tensor.transpose(pst[:], nm[:], ident[:])
                        blk = aTpk[:, (t // 2) * P:(t // 2 + 1) * P]
                        if t % 2 == 0:
                            nc.scalar.copy(blk[0:CM, :], pst[:])
                        else:
                            stg = pa.tile([CM, P], F32, tag="stg")
                            nc.scalar.copy(stg[:], pst[:])
                            nc.sync.dma_start(out=blk[CM:P, :], in_=stg[:])

                    # ---- Phase B: scores + argmax + slot computation ----
                    def rank_and_scatter(i):
                        """Per-tile permutation work: bucket, within-tile rank,
                        cross-tile count, slot, scatter (row, idx) to perm8."""
                        mi8 = mi_all[:, 8 * i:8 * i + 8]
                        nc.vector.tensor_copy(idxf[:, i:i + 1], mi8[:, 0:1])
                        nc.vector.tensor_copy(pr_all[:, 2 * i + 1:2 * i + 2],
                                              mi8[:, 0:1])
                        bu = pb.tile([P, 1], U32, tag="bu")
                        nc.vector.tensor_scalar(bu[:], mi8[:, 0:1], 7, None,
                                                op0=OP.logical_shift_right)
                        bf = pb.tile([P, 1], F32, tag="bf")
                        nc.gpsimd.tensor_copy(bf[:], bu[:])
                        # one-hot over 16 buckets
                        oh = pb.tile([P, 16], F32, tag="oh")
                        nc.vector.tensor_scalar(oh[:], iota16[:], bf[:, 0:1],
                                                None, op0=OP.is_equal)
                        # within-tile rank: W[p] = #{p' < p: b[p'] == b[p]}
                        bt_ps = psr.tile([P, P], F32, tag="bt", space="PSUM")
                        nc.tensor.transpose(bt_ps[:], bf[:].to_broadcast([P, P]),
                                            ident[:])
                        bt_sb = pb.tile([P, P], F32, tag="bts")
                        nc.scalar.copy(bt_sb[:], bt_ps[:])
                        scr = pb.tile([P, P], F32, tag="scr")
                        wcol = pb.tile([P, 1], F32, tag="wc")
                        nc.vector.scalar_tensor_tensor(
                            out=scr[:], in0=bt_sb[:], scalar=bf[:, 0:1],
                            in1=ltm[:], op0=OP.is_equal, op1=OP.mult,
                            accum_out=wcol[:])
                        # cross-tile count of this bucket so far
                        cb = pb.tile([P, 16], F32, tag="cb")
                        nc.gpsimd.partition_broadcast(cb[:], crun[:])
                        ctv = pb.tile([P, 1], F32, tag="ctv")
                        s2 = pb.tile([P, 16], F32, tag="s2")
                        nc.vector.scalar_tensor_tensor(
                            out=s2[:], in0=oh[:], scalar=1.0, in1=cb[:],
                            op0=OP.mult, op1=OP.mult, accum_out=ctv[:])
                        # update running counts: crun += hist(b) (via matmul)
                        hp = psr.tile([1, 16], F32, tag="hp", space="PSUM")
                        nc.tensor.matmul(hp[:], ones128[:, 0:1], oh[:],
                                         start=True, stop=True)
                        nc.vector.tensor_add(crun[:], crun[:], hp[:])
                        # slot = 256*b + ctv + W
                        sf = slot_f[:, i:i + 1]
                        nc.vector.scalar_tensor_tensor(
                            out=sf, in0=bf[:], scalar=256.0, in1=ctv[:],
                            op0=OP.mult, op1=OP.add)
                        nc.vector.tensor_tensor(out=sf, in0=sf, in1=wcol[:],
                                                op=OP.add)
                        nc.vector.tensor_copy(slot_i32[:, i:i + 1], sf)
                        nc.vector.tensor_copy(pr_all[:, 2 * i:2 * i + 1],
                                              xrow_i32[:, i:i + 1])
                        nc.gpsimd.indirect_dma_start(
                            out=perm8[:], in_=pr_all[:, 2 * i:2 * i + 2],
                            in_offset=None,
                            out_offset=IndirectOffsetOnAxis(
                                ap=slot_i32[:, i:i + 1], axis=0))

                    for ii in range(TI // 2):
                        i0, i1 = 2 * ii, 2 * ii + 1
                        ssb0 = pb.tile([P, T1], F32, tag="scores0")
                        ssb1 = pb.tile([P, T1], F32, tag="scores1")
                        for c in range(4):
                            nj = c * 512
                            ps0 = psb.tile([P, 512], F32, tag="ps0", space="PSUM")
                            ps1 = psb.tile([P, 512], F32, tag="ps1", space="PSUM")
                            nc.tensor.matmul(ps0[:], aTpk[0:CM, ii * P:(ii + 1) * P],
                                             bTpk[0:CM, nj:nj + 512],
                                             start=True, stop=True,
                                             tile_position=(0, 0))
                            nc.tensor.matmul(ps1[:], aTpk[CM:P, ii * P:(ii + 1) * P],
                                             bTpk[CM:P, nj:nj + 512],
                                             start=True, stop=True,
                                             tile_position=(64, 0))
                            nc.scalar.copy(ssb0[:, nj:nj + 512], ps0[:])
                            nc.scalar.copy(ssb1[:, nj:nj + 512], ps1[:])
                        for i, ssb in ((i0, ssb0), (i1, ssb1)):
                            mx8 = pb.tile([P, 8], F32, tag="mx")
                            nc.vector.max(out=mx8[:], in_=ssb[:])
                            nc.vector.max_index(out=mi_all[:, 8 * i:8 * i + 8],
                                                in_max=mx8[:], in_values=ssb[:])
                            rank_and_scatter(i)

                    # gather offsets for phase D (odd row of out = 2*idx+1)
                    nc.vector.tensor_copy(offs[:], idxf[:])

            # ---- Phase C: bucketed one-hot scatter matmul ----
            with tc.tile_pool(name="pq", bufs=1) as pqp, \
                 tc.tile_pool(name="pC", bufs=6) as pcs, \
                 tc.tile_pool(name="pD", bufs=2) as pd, \
                 tc.tile_pool(name="psC", bufs=2, space="PSUM") as psc:
                pq = pqp.tile([P, NSLOT // P * 2], I32)
                nc.sync.dma_start(
                    out=pq[:].rearrange("p (u w) -> p u w", w=2), in_=perm_pv)
                pq_v = pq[:].rearrange("p (u w) -> p u w", w=2)
                idxg_f = pqp.tile([P, NSLOT // P], F32)
                nc.vector.tensor_copy(idxg_f[:], pq_v[:, :, 1])
                # clamp pad-slot offsets (BIG) to a valid row; their one-hot
                # row is all-zero so the gathered data is multiplied away
                qoff = pqp.tile([P, NSLOT // P], I32)
                nc.vector.tensor_scalar(qoff[:], pq_v[:, :, 0], T - 2, None,
                                        op0=OP.min)

                ones_r = pqp.tile([P, 2], F32R)
                nc.vector.tensor_copy(ones_r[:], ones128[:, 0:2])
                for jt in range(TJ):
                    psj = psc.tile([P, CX], F32, tag="sp", space="PSUM")
                    psn = psc.tile([P, 2], F32, tag="sn", space="PSUM")
                    for k in range(2):
                        u = 2 * jt + k
                        xg = pcs.tile([P, CX], F32, tag="xg")
                        nc.gpsimd.indirect_dma_start(
                            out=xg[:], out_offset=None,
                            in_=x_in[:],
                            in_offset=IndirectOffsetOnAxis(
                                ap=qoff[:, u:u + 1], axis=0))
                        xh = pcs.tile([P, CX], F32R, tag="xh")
                        xl = pcs.tile([P, CX], F32R, tag="xl")
                        nc.vector.tensor_copy(xh[:], xg[:])
                        nc.vector.tensor_tensor(out=xl[:], in0=xg[:],
                                                in1=xh[:].bitcast(F32),
                                                op=OP.subtract)
                        eqr = pcs.tile([P, P], F32R, tag="eq")
                        nc.vector.scalar_tensor_tensor(
                            out=eqr[:],
                            in0=iota_row[:, jt * P:(jt + 1) * P],
                            scalar=idxg_f[:, u:u + 1],
                            in1=ones128[:],
                            op0=OP.is_equal, op1=OP.mult)
                        first, last = (k == 0), (k == 1)
                        nc.tensor.matmul(psn[:], eqr[:], ones_r[:],
                                         start=first, stop=last)
                        for lo_, hi_ in ((0, 512), (512, CX)):
                            nc.tensor.matmul(psj[:, lo_:hi_], eqr[:],
                                             xh[:, lo_:hi_],
                                             start=first, stop=False)
                            nc.tensor.matmul(psj[:, lo_:hi_], eqr[:],
                                             xl[:, lo_:hi_],
                                             start=False, stop=last)
                    xo = pd.tile([P, CX], F32, tag="xo")
                    nc.sync.dma_start(out=xo[:], in_=x_r[1, jt])
                    cnt1 = pd.tile([P, 1], F32, tag="c1")
                    nc.vector.tensor_scalar_add(cnt1[:], psn[:, 0:1], 1.0)
                    inv = pd.tile([P, 1], F32, tag="iv")
                    nc.vector.reciprocal(inv[:], cnt1[:])
                    dst = pd.tile([P, CX], F32, tag="dst")
                    nc.vector.tensor_add(dst[:], xo[:], psj[:, 0:CX])
                    nc.scalar.mul(dst[:], dst[:], inv[:, 0:1])
                    nc.sync.dma_start(out=out_r[1, jt], in_=dst[:])
                    nc.scalar.dma_start(out=dstbuf[jt * P:(jt + 1) * P, :],
                                        in_=dst[:])

            # ---- Phase D: gather even rows, store in 4-tile chunks ----
            with tc.tile_pool(name="pG", bufs=1) as pg:
                for q in range(TI // 4):
                    gb = pg.tile([P, 4 * CX], F32, name=f"gb{q}", tag=f"gb{q}")
                    for k in range(4):
                        i = 4 * q + k
                        nc.gpsimd.indirect_dma_start(
                            out=gb[:, k * CX:(k + 1) * CX], out_offset=None,
                            in_=dstbuf[:],
                            in_offset=IndirectOffsetOnAxis(ap=offs[:, i:i + 1],
                                                           axis=0))
                    nc.sync.dma_start(
                        out=out_pv[0][:, 4 * q:4 * q + 4, :],
                        in_=gb[:].rearrange("p (t c) -> p t c", c=CX))

    nc.compile()
    return nc


def kernel(metric: np.ndarray, x: np.ndarray) -> np.ndarray:
    if "nc" not in _CACHE:
        _CACHE["nc"] = _build()
    nc = _CACHE["nc"]
    metric = np.ascontiguousarray(np.asarray(metric, dtype=np.float32))
    x = np.ascontiguousarray(np.asarray(x, dtype=np.float32))
    in_maps = [{"metric": metric[c], "x": x[c]} for c in range(N)]
    res = run_bass_kernel_spmd(nc, in_maps, list(range(N)))
    return np.stack([res.results[c]["out"] for c in range(N)], axis=0)
